# revision 3
# baseline (speedup 1.0000x reference)
"""Causal CoreAttention kernel for Trainium2 (Bass/Tile), 8-core SPMD. v2.

Problem: B=2, H=16, S=2048, D=128 fp32 causal attention.
Sharding: B*H=32 heads -> 4 heads per core across 8 cores.

Design:
  - Q^T, K^T uploaded pre-transposed from host (layout-only change): no PE
    transposes, no DVE PSUM evacuations.
  - QK^T strips (f32r, causal-exact) written into a 2-deep ring of 3-bank
    PSUM tiles; ACT evacuates with exp in large contiguous gulps spanning
    strip boundaries (packed P^T layout makes psum ring cols <-> ptall cols
    both contiguous).
  - PV chains (bf16, ones-column denominator trick) interleaved into the PE
    stream one gulp behind, always after the current step's QK so ACT never
    starves; one continuous global stream across heads.
  - DMAs issued from SP (HWDGE), first loads split so gulp 0 starts early;
    output DMA split so the post-ACT tail is short; last head's trailing PV
    chains split so only the last two k-blocks remain after the final exp.
"""
import math

import numpy as np

import concourse.bass as bass
import concourse.mybir as mybir
import concourse.tile as tile
from concourse.bass_utils import run_bass_kernel_spmd
from concourse.masks import make_upper_triangular

B, H, S, D = 2, 16, 2048, 128
NCORES = 8
HPC = (B * H) // NCORES          # heads per core
NT = S // 128                    # 16 q/k tiles per head
SCALE = 1.0 / math.sqrt(D)

MAX_WAITS = 1  # walrus TRN2 encodes at most 1 sync-wait per instruction

# P^T strip packing order (natural; permutations that reduce 128-wide f32r
# chunk penalties were tried but lose more to PV-unlock delays than they
# save in PE cycles).
ORDER = list(range(NT))
OFF = [0] * NT
_t = 0
for _kt in ORDER:
    OFF[_kt] = _t
    _t += S - 128 * _kt
PT_LEN = _t  # 17408


def _split_waits(nc):
    """Tile emits >1 sem-wait on some instructions; hoist extras onto NoOps
    inserted just before, on the same (in-order) engine."""
    for f in nc.m.functions:
        for bb in f.blocks:
            insts = bb.instructions
            out = []
            changed = False
            for inst in insts:
                si = inst.sync_info
                if si is not None and len(si.on_wait) > MAX_WAITS:
                    waits = list(si.on_wait)
                    extra, keep = waits[:-MAX_WAITS], waits[-MAX_WAITS:]
                    for j in range(0, len(extra), MAX_WAITS):
                        nop = mybir.InstNoOp(
                            name=f"{inst.name}-ws{j}", engine=inst.engine)
                        nop.sync_info = mybir.SyncInfo(
                            on_wait=extra[j:j + MAX_WAITS], on_update=[])
                        out.append(nop)
                    inst.sync_info = mybir.SyncInfo(
                        on_wait=keep, on_update=list(si.on_update))
                    changed = True
                out.append(inst)
            if changed:
                insts[:] = out


def _plan(gw, fine_tail=False, fine_start=False):
    """Gulp plan: returns (bounds, gulps, g_mask, g_pv). bounds = packed-col
    gulp boundaries; gulps[g] = [(kt, packed_lo, packed_hi)] QK chunks cut at
    the 512 PSUM bank grid and at strip/gulp boundaries. fine_tail splits the
    final stretch (1024/1024/512/512/512) so PV chains unlock progressively;
    fine_start splits gulp 0 (512/1024) so the first exp starts sooner."""
    assert gw % 512 == 0
    bounds = list(range(0, PT_LEN, gw))
    if fine_tail:
        # fine_tail = tuple of tail gulp sizes covering the last stretch
        coarse = [b for b in bounds if b <= PT_LEN - 2 * gw]
        b0 = coarse[-1] + gw
        assert sum(fine_tail) == PT_LEN - b0, (fine_tail, PT_LEN - b0)
        bounds = coarse + [b0]
        for sz in fine_tail[:-1]:
            bounds.append(bounds[-1] + sz)
    if fine_start:
        bounds = sorted(set(bounds) | {512})
    bounds.append(PT_LEN)
    ng = len(bounds) - 1

    def gulp_of(pos):
        for g in range(ng):
            if bounds[g] <= pos < bounds[g + 1]:
                return g
        raise AssertionError(pos)

    cuts = set(OFF)
    cuts.update(range(0, PT_LEN + 1, 512))
    cuts.update(bounds)
    cuts = sorted(c for c in cuts if c <= PT_LEN)
    gulps = [[] for _ in range(ng)]

    def strip_of(pos):
        for kt in range(NT):
            if OFF[kt] <= pos < OFF[kt] + (S - 128 * kt):
                return kt
        raise AssertionError(pos)

    for lo, hi in zip(cuts[:-1], cuts[1:]):
        kt = strip_of(lo)
        g = gulp_of(lo)
        assert gulp_of(hi - 1) == g
        # chunk must stay within one psum bank (ring tiles are bank-aligned,
        # gulp starts are 512-aligned)
        assert (lo - bounds[g]) // 512 == (hi - 1 - bounds[g]) // 512
        gulps[g].append((kt, lo, hi))
    g_mask = [gulp_of(OFF[kt] + 127) for kt in range(NT)]
    g_pv = [gulp_of(OFF[qt] + 127) for qt in range(NT)]
    return bounds, gulps, g_mask, g_pv


def build_nc(gw=1536, pvw=129, lag=1, prefetch_g=2, budget_mm=10, warm=2,
             fine_tail=None, fine_start_all=False, tail_split=0,
             tail_act_dma=False, prio_first_loads=False):
    fp32 = mybir.dt.float32
    f32r = mybir.dt.float32r
    bf16 = mybir.dt.bfloat16

    plan_0 = _plan(gw, fine_start=True)
    plan_a = _plan(gw, fine_start=fine_start_all)
    plan_b = _plan(gw, fine_tail=fine_tail, fine_start=fine_start_all)
    plans = [plan_0] + [plan_a] * (HPC - 2) + [plan_b]
    step_base = [0]
    for h in range(HPC):
        step_base.append(step_base[-1] + len(plans[h][1]))

    nc = bass.Bass("TRN2", target_bir_lowering=False)
    # qT/kT uploaded [head, d, s] (host-transposed); v natural [head, s, d]
    qT = nc.dram_tensor("qT", [HPC, D, S], f32r, kind="ExternalInput").ap()
    kT = nc.dram_tensor("kT", [HPC, D, S], f32r, kind="ExternalInput").ap()
    v = nc.dram_tensor("v", [HPC, S, D], fp32, kind="ExternalInput").ap()
    o = nc.dram_tensor("o", [HPC, S, D], fp32, kind="ExternalOutput").ap()

    with tile.TileContext(nc) as tc:
        with tc.tile_pool(name="const", bufs=1) as constp, \
             tc.tile_pool(name="nat", bufs=2) as natp, \
             tc.tile_pool(name="pt", bufs=2) as ptp, \
             tc.tile_pool(name="osb", bufs=2) as osbp, \
             tc.tile_pool(name="rc", bufs=2) as rcp, \
             tc.tile_pool(name="qk_ps", bufs=2, space="PSUM") as qkps, \
             tc.tile_pool(name="pv_ps", bufs=2, space="PSUM") as pvps:

            ltri = constp.tile([128, 128], bf16, tag="ltri")
            # keep P^T[k,q] where k <= q (partition <= free)
            make_upper_triangular(nc, ltri[:], val=1.0, diag=True)

            # PE warmup: dummy matmuls while the first loads land, so the
            # first real QK chunks run at a ramped pstate
            if warm:
                wsrc = constp.tile([128, 512], bf16, tag="wsrc")
                wsnk = constp.tile([128, 1], fp32, tag="wsnk")
                nc.vector.memset(wsrc[:], 0.0)
                wps = pvps.tile([128, pvw], fp32, tag="pv")
                for _ in range(warm):
                    nc.tensor.matmul(wps[:], wsrc[:, 0:128], wsrc[:, 0:pvw],
                                     start=True, stop=True)
                # dummy reader keeps the BIR verifier happy
                nc.vector.tensor_copy(wsnk[:], wps[:, 0:1])

            st = {}  # per-head tiles

            def ensure_loaded(h):
                if h >= HPC or h in st:
                    return
                qTt = natp.tile([128, S], f32r, tag="qT")
                kTt = natp.tile([128, S], f32r, tag="kT")
                vn = natp.tile([128, NT, 128], fp32, tag="vn")
                va = natp.tile([128, NT, pvw + 1], bf16, tag="va")
                # split loads, ordered by first use; head 0 is latency
                # critical (others prefetch a head ahead)
                if h == 0:
                    # first pieces via the ACT hwdge queue (idle at startup,
                    # and SP's 650ns-per-issue cadence would gate them)
                    import contextlib
                    hp = (tc.high_priority() if prio_first_loads
                          else contextlib.nullcontext())
                    with hp:
                        nc.scalar.dma_start(kTt[:, 0:128], kT[h][:, 0:128])
                        nc.scalar.dma_start(qTt[:, 0:512], qT[h][:, 0:512])
                    nc.sync.dma_start(qTt[:, 512:1024], qT[h][:, 512:1024])
                    nc.sync.dma_start(
                        qTt[:, 1024:1536], qT[h][:, 1024:1536])
                    nc.sync.dma_start(kTt[:, 128:512], kT[h][:, 128:512])
                    nc.sync.dma_start(
                        qTt[:, 1536:2048], qT[h][:, 1536:2048])
                    nc.sync.dma_start(vn[:], v[h].rearrange(
                        "(t p) d -> p t d", p=128))
                    nc.sync.dma_start(kTt[:, 512:S], kT[h][:, 512:S])
                else:
                    nc.sync.dma_start(kTt[:, 0:512], kT[h][:, 0:512])
                    nc.sync.dma_start(qTt[:], qT[h])
                    nc.sync.dma_start(vn[:], v[h].rearrange(
                        "(t p) d -> p t d", p=128))
                    nc.sync.dma_start(kTt[:, 512:S], kT[h][:, 512:S])
                # V -> bf16 + ones/pad columns (DVE)
                nc.vector.memset(va[:, :, 128:], 1.0)
                nc.vector.tensor_copy(va[:, :, 0:128], vn[:])
                ptt = ptp.tile([128, PT_LEN], bf16, tag="pt")
                osb = osbp.tile([128, NT, 128], fp32, tag="osb")
                rc = rcp.tile([128, NT], fp32, tag="rc")
                st[h] = dict(qT=qTt, kT=kTt, va=va, pt=ptt, osb=osb, rc=rc)

            def emit_pv_part(h, qt, kt_lo, kt_hi, po):
                """Accumulate P^T[kt_lo..kt_hi] @ Vaug into psum region po;
                on the final part (kt_hi == qt) normalize and store/DMA."""
                s = st[h]
                for kt in range(kt_lo, kt_hi + 1):
                    blk = OFF[kt] + (qt - kt) * 128
                    nc.tensor.matmul(
                        po,
                        s["pt"][:, blk:blk + 128],
                        s["va"][:, kt, 0:pvw],
                        start=(kt == 0), stop=(kt == qt))
                if kt_hi < qt:
                    return
                nc.vector.reciprocal(
                    s["rc"][:, qt:qt + 1], po[:, 128:129])
                nc.vector.tensor_scalar_mul(
                    s["osb"][:, qt, :], po[:, 0:128], s["rc"][:, qt:qt + 1])
                # output DMA: bulk for qt<=7 at qt==7, then per-tile pieces
                # (keeps the store off the critical tail)
                orr = o[h].rearrange("(t p) d -> p t d", p=128)
                eng = (nc.scalar if tail_act_dma and h == HPC - 1 and qt >= 14
                       else nc.sync)
                if qt == 7:
                    nc.sync.dma_start(orr[:, 0:8], s["osb"][:, 0:8])
                elif qt > 7:
                    eng.dma_start(orr[:, qt:qt + 1], s["osb"][:, qt:qt + 1])

            def emit_pv(h, qt):
                po = pvps.tile([128, pvw], fp32, tag="pv")
                emit_pv_part(h, qt, 0, qt, po[:])

            ensure_loaded(0)
            ensure_loaded(1)

            # global pipelined stream over (head, gulp); QK runs one step
            # ahead of exp/drain emission so ACT is never starved, chains
            # drain under a per-step budget
            pend = []  # (due_T, h, qt)
            for h in range(HPC):
                g_pv_h = plans[h][3]
                hlag = 0 if h == HPC - 1 else lag
                for qt in range(NT):
                    pend.append((step_base[h] + g_pv_h[qt] + hlag, h, qt))
            pend.sort()
            pi = 0
            ready = []

            steps = []
            for h in range(HPC):
                bounds, gulps, g_mask, _ = plans[h]
                for g in range(len(gulps)):
                    steps.append((h, g))
            rings = {}

            def emit_qk(h, g):
                bounds, gulps, _, _ = plans[h]
                s = st[h]
                qTr = s["qT"][:]
                kTr = s["kT"][:]
                g0 = bounds[g]
                ring = qkps.tile([128, gw], fp32, tag="qk")
                for (kt, lo, hi) in gulps[g]:
                    q_a = 128 * kt + (lo - OFF[kt])
                    nc.tensor.matmul(
                        ring[:, lo - g0:hi - g0],
                        kTr[:, 128 * kt:128 * kt + 128],
                        qTr[:, q_a:q_a + (hi - lo)],
                        start=True, stop=True)
                return ring

            emitted = set()
            for i, (h, g) in enumerate(steps):
                T = i
                bounds, gulps, g_mask, _ = plans[h]
                ng = len(gulps)
                s = st[h]
                if g == prefetch_g:
                    ensure_loaded(h + 1)
                if i not in emitted:
                    rings[i] = emit_qk(h, g)
                    emitted.add(i)
                # boundary lookahead: when this is a head's last gulp, emit
                # the next head's first QK before draining tail chains so
                # ACT rolls straight into the next head
                if g == ng - 1 and i + 1 < len(steps):
                    hn, gn = steps[i + 1]
                    ensure_loaded(hn)
                    rings[i + 1] = emit_qk(hn, gn)
                    emitted.add(i + 1)
                ring = rings.pop(i)
                g0, g1 = bounds[g], bounds[g + 1]
                nc.scalar.activation(
                    s["pt"][:, g0:g1], ring[:, 0:g1 - g0],
                    mybir.ActivationFunctionType.Exp, scale=SCALE)
                for kt in range(NT):
                    if g_mask[kt] == g:
                        nc.vector.tensor_mul(
                            s["pt"][:, OFF[kt]:OFF[kt] + 128],
                            s["pt"][:, OFF[kt]:OFF[kt] + 128],
                            ltri[:])
                while pi < len(pend) and pend[pi][0] <= T - 1:
                    ready.append(pend[pi][1:])
                    pi += 1
                eager = h == HPC - 1 and g >= ng - 4
                bud = 10 ** 9 if eager else budget_mm
                while ready and bud > 0:
                    h_, qt_ = ready.pop(0)
                    emit_pv(h_, qt_)
                    bud -= qt_ + 1
            # flush remaining PVs
            rest = ready + [pend[p][1:] for p in range(pi, len(pend))]
            if tail_split:
                # split the last chains: accumulate k-blocks evacuated by
                # earlier gulps now; finish (and normalize) after the final
                # exp with only the last k-blocks
                parts = []
                whole = []
                for (h_, qt_) in rest:
                    if h_ == HPC - 1 and qt_ >= NT - tail_split:
                        kt_mid = qt_ - 2
                        po = pvps.tile([128, pvw], fp32, tag="pv")
                        parts.append((h_, qt_, kt_mid, po))
                    else:
                        whole.append((h_, qt_))
                for (h_, qt_) in whole:
                    emit_pv(h_, qt_)
                for (h_, qt_, kt_mid, po) in parts:
                    emit_pv_part(h_, qt_, 0, kt_mid, po[:])
                for (h_, qt_, kt_mid, po) in parts:
                    emit_pv_part(h_, qt_, kt_mid + 1, qt_, po[:])
            else:
                for (h_, qt_) in rest:
                    emit_pv(h_, qt_)

    _split_waits(nc)
    return nc


_NC = None


def kernel(query_states, key_states, value_states):
    global _NC
    qf = np.asarray(query_states, dtype=np.float32).reshape(B * H, S, D)
    kf = np.asarray(key_states, dtype=np.float32).reshape(B * H, S, D)
    vf = np.ascontiguousarray(
        np.asarray(value_states, dtype=np.float32).reshape(B * H, S, D))
    qT = np.ascontiguousarray(qf.transpose(0, 2, 1))
    kT = np.ascontiguousarray(kf.transpose(0, 2, 1))

    if _NC is None:
        _NC = build_nc()

    in_maps = [
        {"qT": qT[i * HPC:(i + 1) * HPC],
         "kT": kT[i * HPC:(i + 1) * HPC],
         "v": vf[i * HPC:(i + 1) * HPC]}
        for i in range(NCORES)
    ]
    res = run_bass_kernel_spmd(_NC, in_maps, core_ids=list(range(NCORES)))
    out = np.concatenate([res.results[i]["o"] for i in range(NCORES)], axis=0)
    return out.reshape(B, H, S, D)


# revision 4
# speedup vs baseline: 1.0023x; 1.0023x over previous
"""Causal CoreAttention kernel for Trainium2 (Bass/Tile), 8-core SPMD. v2.

Problem: B=2, H=16, S=2048, D=128 fp32 causal attention.
Sharding: B*H=32 heads -> 4 heads per core across 8 cores.

Design:
  - Q^T, K^T uploaded pre-transposed from host (layout-only change): no PE
    transposes, no DVE PSUM evacuations.
  - QK^T strips (f32r, causal-exact) written into a 2-deep ring of 3-bank
    PSUM tiles; ACT evacuates with exp in large contiguous gulps spanning
    strip boundaries (packed P^T layout makes psum ring cols <-> ptall cols
    both contiguous).
  - PV chains (bf16, ones-column denominator trick) interleaved into the PE
    stream one gulp behind, always after the current step's QK so ACT never
    starves; one continuous global stream across heads.
  - DMAs issued from SP (HWDGE), first loads split so gulp 0 starts early;
    output DMA split so the post-ACT tail is short; last head's trailing PV
    chains split so only the last two k-blocks remain after the final exp.
"""
import math

import numpy as np

import concourse.bass as bass
import concourse.mybir as mybir
import concourse.tile as tile
from concourse.bass_utils import run_bass_kernel_spmd
from concourse.masks import make_upper_triangular

B, H, S, D = 2, 16, 2048, 128
NCORES = 8
HPC = (B * H) // NCORES          # heads per core
NT = S // 128                    # 16 q/k tiles per head
SCALE = 1.0 / math.sqrt(D)

MAX_WAITS = 1  # walrus TRN2 encodes at most 1 sync-wait per instruction

# P^T strip packing order (natural; permutations that reduce 128-wide f32r
# chunk penalties were tried but lose more to PV-unlock delays than they
# save in PE cycles).
ORDER = list(range(NT))
OFF = [0] * NT
_t = 0
for _kt in ORDER:
    OFF[_kt] = _t
    _t += S - 128 * _kt
PT_LEN = _t  # 17408


def _split_waits(nc):
    """Tile emits >1 sem-wait on some instructions; hoist extras onto NoOps
    inserted just before, on the same (in-order) engine."""
    for f in nc.m.functions:
        for bb in f.blocks:
            insts = bb.instructions
            out = []
            changed = False
            for inst in insts:
                si = inst.sync_info
                if si is not None and len(si.on_wait) > MAX_WAITS:
                    waits = list(si.on_wait)
                    extra, keep = waits[:-MAX_WAITS], waits[-MAX_WAITS:]
                    for j in range(0, len(extra), MAX_WAITS):
                        nop = mybir.InstNoOp(
                            name=f"{inst.name}-ws{j}", engine=inst.engine)
                        nop.sync_info = mybir.SyncInfo(
                            on_wait=extra[j:j + MAX_WAITS], on_update=[])
                        out.append(nop)
                    inst.sync_info = mybir.SyncInfo(
                        on_wait=keep, on_update=list(si.on_update))
                    changed = True
                out.append(inst)
            if changed:
                insts[:] = out


def _plan(gw, fine_tail=False, fine_start=False):
    """Gulp plan: returns (bounds, gulps, g_mask, g_pv). bounds = packed-col
    gulp boundaries; gulps[g] = [(kt, packed_lo, packed_hi)] QK chunks cut at
    the 512 PSUM bank grid and at strip/gulp boundaries. fine_tail splits the
    final stretch (1024/1024/512/512/512) so PV chains unlock progressively;
    fine_start splits gulp 0 (512/1024) so the first exp starts sooner."""
    assert gw % 512 == 0
    bounds = list(range(0, PT_LEN, gw))
    if fine_tail:
        # fine_tail = tuple of tail gulp sizes covering the last stretch
        coarse = [b for b in bounds if b <= PT_LEN - 2 * gw]
        b0 = coarse[-1] + gw
        assert sum(fine_tail) == PT_LEN - b0, (fine_tail, PT_LEN - b0)
        bounds = coarse + [b0]
        for sz in fine_tail[:-1]:
            bounds.append(bounds[-1] + sz)
    if fine_start:
        bounds = sorted(set(bounds) | set(fine_start))
    bounds.append(PT_LEN)
    ng = len(bounds) - 1

    def gulp_of(pos):
        for g in range(ng):
            if bounds[g] <= pos < bounds[g + 1]:
                return g
        raise AssertionError(pos)

    cuts = set(OFF)
    cuts.update(range(0, PT_LEN + 1, 512))
    cuts.update(bounds)
    cuts = sorted(c for c in cuts if c <= PT_LEN)
    gulps = [[] for _ in range(ng)]

    def strip_of(pos):
        for kt in range(NT):
            if OFF[kt] <= pos < OFF[kt] + (S - 128 * kt):
                return kt
        raise AssertionError(pos)

    for lo, hi in zip(cuts[:-1], cuts[1:]):
        kt = strip_of(lo)
        g = gulp_of(lo)
        assert gulp_of(hi - 1) == g
        # chunk must stay within one psum bank (ring tiles are bank-aligned,
        # gulp starts are 512-aligned)
        assert (lo - bounds[g]) // 512 == (hi - 1 - bounds[g]) // 512
        gulps[g].append((kt, lo, hi))
    g_mask = [gulp_of(OFF[kt] + 127) for kt in range(NT)]
    g_pv = [gulp_of(OFF[qt] + 127) for qt in range(NT)]
    return bounds, gulps, g_mask, g_pv


def build_nc(gw=1536, pvw=129, lag=1, prefetch_g=2, budget_mm=10, warm=2,
             fine_tail=None, fine_start=(512, 1024, 2048), tail_split=0,
             tail_act_dma=False, prio_first_loads=False, qt0_sp=True):
    fp32 = mybir.dt.float32
    f32r = mybir.dt.float32r
    bf16 = mybir.dt.bfloat16

    plan_0 = _plan(gw, fine_start=fine_start)
    plan_a = _plan(gw)
    plan_b = _plan(gw, fine_tail=fine_tail)
    plans = [plan_0] + [plan_a] * (HPC - 2) + [plan_b]
    step_base = [0]
    for h in range(HPC):
        step_base.append(step_base[-1] + len(plans[h][1]))

    nc = bass.Bass("TRN2", target_bir_lowering=False)
    # qT/kT uploaded [head, d, s] (host-transposed); v natural [head, s, d]
    qT = nc.dram_tensor("qT", [HPC, D, S], f32r, kind="ExternalInput").ap()
    kT = nc.dram_tensor("kT", [HPC, D, S], f32r, kind="ExternalInput").ap()
    v = nc.dram_tensor("v", [HPC, S, D], fp32, kind="ExternalInput").ap()
    o = nc.dram_tensor("o", [HPC, S, D], fp32, kind="ExternalOutput").ap()

    with tile.TileContext(nc) as tc:
        with tc.tile_pool(name="const", bufs=1) as constp, \
             tc.tile_pool(name="nat", bufs=2) as natp, \
             tc.tile_pool(name="pt", bufs=2) as ptp, \
             tc.tile_pool(name="osb", bufs=2) as osbp, \
             tc.tile_pool(name="rc", bufs=2) as rcp, \
             tc.tile_pool(name="qk_ps", bufs=2, space="PSUM") as qkps, \
             tc.tile_pool(name="pv_ps", bufs=2, space="PSUM") as pvps:

            ltri = constp.tile([128, 128], bf16, tag="ltri")
            # keep P^T[k,q] where k <= q (partition <= free)
            make_upper_triangular(nc, ltri[:], val=1.0, diag=True)

            # PE warmup: dummy matmuls while the first loads land, so the
            # first real QK chunks run at a ramped pstate
            if warm:
                wsrc = constp.tile([128, 512], bf16, tag="wsrc")
                wsnk = constp.tile([128, 1], fp32, tag="wsnk")
                nc.vector.memset(wsrc[:], 0.0)
                wps = pvps.tile([128, pvw], fp32, tag="pv")
                for _ in range(warm):
                    nc.tensor.matmul(wps[:], wsrc[:, 0:128], wsrc[:, 0:pvw],
                                     start=True, stop=True)
                # dummy reader keeps the BIR verifier happy
                nc.vector.tensor_copy(wsnk[:], wps[:, 0:1])

            st = {}  # per-head tiles

            def ensure_loaded(h):
                if h >= HPC or h in st:
                    return
                qTt = natp.tile([128, S], f32r, tag="qT")
                kTt = natp.tile([128, S], f32r, tag="kT")
                vn = natp.tile([128, NT, 128], fp32, tag="vn")
                va = natp.tile([128, NT, pvw + 1], bf16, tag="va")
                # split loads, ordered by first use; head 0 is latency
                # critical (others prefetch a head ahead)
                if h == 0:
                    # first pieces via the ACT hwdge queue (idle at startup,
                    # and SP's 650ns-per-issue cadence would gate them)
                    import contextlib
                    hp = (tc.high_priority() if prio_first_loads
                          else contextlib.nullcontext())
                    with hp:
                        if qt0_sp:
                            nc.sync.dma_start(
                                qTt[:, 0:512], qT[h][:, 0:512])
                            nc.scalar.dma_start(
                                kTt[:, 0:128], kT[h][:, 0:128])
                        else:
                            nc.scalar.dma_start(
                                kTt[:, 0:128], kT[h][:, 0:128])
                            nc.scalar.dma_start(
                                qTt[:, 0:512], qT[h][:, 0:512])
                    nc.sync.dma_start(qTt[:, 512:1024], qT[h][:, 512:1024])
                    nc.sync.dma_start(
                        qTt[:, 1024:1536], qT[h][:, 1024:1536])
                    nc.sync.dma_start(kTt[:, 128:512], kT[h][:, 128:512])
                    nc.sync.dma_start(
                        qTt[:, 1536:2048], qT[h][:, 1536:2048])
                    nc.sync.dma_start(kTt[:, 512:1024], kT[h][:, 512:1024])
                    nc.sync.dma_start(vn[:], v[h].rearrange(
                        "(t p) d -> p t d", p=128))
                    nc.sync.dma_start(kTt[:, 1024:S], kT[h][:, 1024:S])
                else:
                    nc.sync.dma_start(kTt[:, 0:512], kT[h][:, 0:512])
                    nc.sync.dma_start(qTt[:], qT[h])
                    nc.sync.dma_start(vn[:], v[h].rearrange(
                        "(t p) d -> p t d", p=128))
                    nc.sync.dma_start(kTt[:, 512:S], kT[h][:, 512:S])
                # V -> bf16 + ones/pad columns (DVE)
                nc.vector.memset(va[:, :, 128:], 1.0)
                nc.vector.tensor_copy(va[:, :, 0:128], vn[:])
                ptt = ptp.tile([128, PT_LEN], bf16, tag="pt")
                osb = osbp.tile([128, NT, 128], fp32, tag="osb")
                rc = rcp.tile([128, NT], fp32, tag="rc")
                st[h] = dict(qT=qTt, kT=kTt, va=va, pt=ptt, osb=osb, rc=rc)

            def emit_pv_part(h, qt, kt_lo, kt_hi, po):
                """Accumulate P^T[kt_lo..kt_hi] @ Vaug into psum region po;
                on the final part (kt_hi == qt) normalize and store/DMA."""
                s = st[h]
                for kt in range(kt_lo, kt_hi + 1):
                    blk = OFF[kt] + (qt - kt) * 128
                    nc.tensor.matmul(
                        po,
                        s["pt"][:, blk:blk + 128],
                        s["va"][:, kt, 0:pvw],
                        start=(kt == 0), stop=(kt == qt))
                if kt_hi < qt:
                    return
                nc.vector.reciprocal(
                    s["rc"][:, qt:qt + 1], po[:, 128:129])
                nc.vector.tensor_scalar_mul(
                    s["osb"][:, qt, :], po[:, 0:128], s["rc"][:, qt:qt + 1])
                # output DMA: bulk for qt<=7 at qt==7, then per-tile pieces
                # (keeps the store off the critical tail)
                orr = o[h].rearrange("(t p) d -> p t d", p=128)
                eng = (nc.scalar if tail_act_dma and h == HPC - 1 and qt >= 14
                       else nc.sync)
                if qt == 7:
                    nc.sync.dma_start(orr[:, 0:8], s["osb"][:, 0:8])
                elif qt > 7:
                    eng.dma_start(orr[:, qt:qt + 1], s["osb"][:, qt:qt + 1])

            def emit_pv(h, qt):
                po = pvps.tile([128, pvw], fp32, tag="pv")
                emit_pv_part(h, qt, 0, qt, po[:])

            ensure_loaded(0)
            ensure_loaded(1)

            # global pipelined stream over (head, gulp); QK runs one step
            # ahead of exp/drain emission so ACT is never starved, chains
            # drain under a per-step budget
            pend = []  # (due_T, h, qt)
            for h in range(HPC):
                g_pv_h = plans[h][3]
                hlag = 0 if h == HPC - 1 else lag
                for qt in range(NT):
                    pend.append((step_base[h] + g_pv_h[qt] + hlag, h, qt))
            pend.sort()
            pi = 0
            ready = []

            steps = []
            for h in range(HPC):
                bounds, gulps, g_mask, _ = plans[h]
                for g in range(len(gulps)):
                    steps.append((h, g))
            rings = {}

            def emit_qk(h, g):
                bounds, gulps, _, _ = plans[h]
                s = st[h]
                qTr = s["qT"][:]
                kTr = s["kT"][:]
                g0 = bounds[g]
                ring = qkps.tile([128, gw], fp32, tag="qk")
                for (kt, lo, hi) in gulps[g]:
                    q_a = 128 * kt + (lo - OFF[kt])
                    nc.tensor.matmul(
                        ring[:, lo - g0:hi - g0],
                        kTr[:, 128 * kt:128 * kt + 128],
                        qTr[:, q_a:q_a + (hi - lo)],
                        start=True, stop=True)
                return ring

            emitted = set()
            for i, (h, g) in enumerate(steps):
                T = i
                bounds, gulps, g_mask, _ = plans[h]
                ng = len(gulps)
                s = st[h]
                if g == prefetch_g:
                    ensure_loaded(h + 1)
                if i not in emitted:
                    rings[i] = emit_qk(h, g)
                    emitted.add(i)
                # boundary lookahead: when this is a head's last gulp, emit
                # the next head's first QK before draining tail chains so
                # ACT rolls straight into the next head
                if g == ng - 1 and i + 1 < len(steps):
                    hn, gn = steps[i + 1]
                    ensure_loaded(hn)
                    rings[i + 1] = emit_qk(hn, gn)
                    emitted.add(i + 1)
                ring = rings.pop(i)
                g0, g1 = bounds[g], bounds[g + 1]
                nc.scalar.activation(
                    s["pt"][:, g0:g1], ring[:, 0:g1 - g0],
                    mybir.ActivationFunctionType.Exp, scale=SCALE)
                for kt in range(NT):
                    if g_mask[kt] == g:
                        nc.vector.tensor_mul(
                            s["pt"][:, OFF[kt]:OFF[kt] + 128],
                            s["pt"][:, OFF[kt]:OFF[kt] + 128],
                            ltri[:])
                while pi < len(pend) and pend[pi][0] <= T - 1:
                    ready.append(pend[pi][1:])
                    pi += 1
                eager = h == HPC - 1 and g >= ng - 4
                bud = 10 ** 9 if eager else budget_mm
                while ready and bud > 0:
                    h_, qt_ = ready.pop(0)
                    emit_pv(h_, qt_)
                    bud -= qt_ + 1
            # flush remaining PVs
            rest = ready + [pend[p][1:] for p in range(pi, len(pend))]
            if tail_split:
                # split the last chains: accumulate k-blocks evacuated by
                # earlier gulps now; finish (and normalize) after the final
                # exp with only the last k-blocks
                parts = []
                whole = []
                for (h_, qt_) in rest:
                    if h_ == HPC - 1 and qt_ >= NT - tail_split:
                        kt_mid = qt_ - 2
                        po = pvps.tile([128, pvw], fp32, tag="pv")
                        parts.append((h_, qt_, kt_mid, po))
                    else:
                        whole.append((h_, qt_))
                for (h_, qt_) in whole:
                    emit_pv(h_, qt_)
                for (h_, qt_, kt_mid, po) in parts:
                    emit_pv_part(h_, qt_, 0, kt_mid, po[:])
                for (h_, qt_, kt_mid, po) in parts:
                    emit_pv_part(h_, qt_, kt_mid + 1, qt_, po[:])
            else:
                for (h_, qt_) in rest:
                    emit_pv(h_, qt_)

    _split_waits(nc)
    return nc


_NC = None


def kernel(query_states, key_states, value_states):
    global _NC
    qf = np.asarray(query_states, dtype=np.float32).reshape(B * H, S, D)
    kf = np.asarray(key_states, dtype=np.float32).reshape(B * H, S, D)
    vf = np.ascontiguousarray(
        np.asarray(value_states, dtype=np.float32).reshape(B * H, S, D))
    qT = np.ascontiguousarray(qf.transpose(0, 2, 1))
    kT = np.ascontiguousarray(kf.transpose(0, 2, 1))

    if _NC is None:
        _NC = build_nc()

    in_maps = [
        {"qT": qT[i * HPC:(i + 1) * HPC],
         "kT": kT[i * HPC:(i + 1) * HPC],
         "v": vf[i * HPC:(i + 1) * HPC]}
        for i in range(NCORES)
    ]
    res = run_bass_kernel_spmd(_NC, in_maps, core_ids=list(range(NCORES)))
    out = np.concatenate([res.results[i]["o"] for i in range(NCORES)], axis=0)
    return out.reshape(B, H, S, D)
